# revision 22
# baseline (speedup 1.0000x reference)
"""Trainium2 Bass kernel for a pre-RMSNorm attention+FFN transformer block.

Problem: x (2, 1024, 4096) fp32, channel-major (B, C, T).
  h = x^T; h += Attn(RMSNorm(h)); h += FFN(RMSNorm(h)); return h^T.

Sharding: 8 cores = 2 batches x 4 query-token chunks of 1024.  Each core
computes K/V for its batch's own 1024-token chunk, AllGathers K/V across
the 4-core replica group, then runs attention + Wo + FFN for its chunk.

All big matmuls run in fp8e4 with DoubleRow perf mode (256-deep
contraction per instruction, 0.5 cycles/row).  Weights are prescaled on
the host (x32 for Wq/Wk/Wv, x64 for Wo/W1/W2) so fp8's normal range is
used; the scales are folded back via activation scale factors and fused
scalar_tensor_tensor residual adds.  The softmax denominator comes from a
DoubleRow matmul against a constant tile; part of the exp work runs as a
Schraudolph bit-trick exp on DVE+Pool to offload the Act engine.
The residual path stays fp32 end to end.
"""

import numpy as np
import ml_dtypes

import concourse.bass as bass
import concourse.mybir as mybir
import concourse.tile as tile
from concourse import bacc
from concourse.bass_utils import run_bass_kernel_spmd

F32 = mybir.dt.float32
BF16 = mybir.dt.bfloat16
FP8 = mybir.dt.float8e4
I32 = mybir.dt.int32
AF = mybir.ActivationFunctionType
ALU = mybir.AluOpType
DRW = mybir.MatmulPerfMode.DoubleRow

B = 2
C = 1024
T = 4096
TQ = 1024          # query-token chunk per core
H = 4
DH = 256
FF = 1536
P = 128
NT = 512           # moving-operand / PSUM tile width
CT = C // P        # 8 channel tiles
TQT = TQ // NT     # 2 chunk token tiles
DB = C // P        # 8 output-channel blocks for q/k/v/o
FFB = FF // P      # 12 ff blocks
TJ = T // P        # 32 key-token blocks
JP = TJ // 2       # 16 key-block pairs

WS_QKV = 32.0      # host prescale on Wq/Wk/Wv
WS = 64.0          # host prescale on Wo/W1/W2
OSC = 16.0         # scale of oT relative to true attention output
ONES_DEN = WS_QKV / OSC              # memset value for the denominator matmul
EXP_SCALE = (DH ** -0.5) / (WS_QKV * WS_QKV)
SCH_A = 12102203.161561485           # 2^23 / ln 2
SCH_B = 127.0 * (1 << 23) - 366000.0
SCH_PAIRS = 6      # key-block pairs per (h, ti) routed to Schraudolph exp

_CACHE = {}
DEBUG = False


def _build():
    nc = bacc.Bacc()
    xq = nc.dram_tensor("xq", [C, TQ], F32, kind="ExternalInput")
    xqb = nc.dram_tensor("xqb", [C, TQ], BF16, kind="ExternalInput")
    wq = nc.dram_tensor("wq", [C, C], FP8, kind="ExternalInput")
    wk = nc.dram_tensor("wk", [C, C], FP8, kind="ExternalInput")
    wv = nc.dram_tensor("wv", [C, C], FP8, kind="ExternalInput")
    wo = nc.dram_tensor("wo", [C, C], FP8, kind="ExternalInput")
    w1 = nc.dram_tensor("w1", [C, FF], FP8, kind="ExternalInput")
    w2 = nc.dram_tensor("w2", [FF, C], BF16, kind="ExternalInput")
    out = nc.dram_tensor("out", [C, TQ], F32, kind="ExternalOutput")
    if DEBUG:
        dbg_q = nc.dram_tensor("dbg_q", [P, DB * TQ], FP8, kind="ExternalOutput")
        dbg_k = nc.dram_tensor("dbg_k", [P, DB * T], FP8, kind="ExternalOutput")
        dbg_v = nc.dram_tensor("dbg_v", [P, TJ * C], FP8, kind="ExternalOutput")
        dbg_e = nc.dram_tensor("dbg_e", [P, TJ * NT], FP8, kind="ExternalOutput")
        dbg_r = nc.dram_tensor("dbg_r", [P, NT], F32, kind="ExternalOutput")
        dbg_o = nc.dram_tensor("dbg_o", [P, DB * TQ], FP8, kind="ExternalOutput")

    RG = [[0, 1, 2, 3], [4, 5, 6, 7]]

    with tile.TileContext(nc) as tc:
        cpool_cm = tc.tile_pool(name="const", bufs=1)
        cpool = cpool_cm.__enter__()
        ones8 = cpool.tile([P, 2, P], FP8, tag="ones8", name="ones8")
        nc.vector.memset(ones8[:], 1.0)
        ones_d = cpool.tile([P, 2, P], FP8, tag="ones_d", name="ones_d")
        nc.vector.memset(ones_d[:], ONES_DEN)
        eps_t = cpool.tile([P, 1], F32, tag="eps", name="eps_t")
        nc.vector.memset(eps_t[:], 1e-8)

        dram_cm = tc.tile_pool(name="dram", bufs=1, space="DRAM")
        dp = dram_cm.__enter__()
        kl_d = dp.tile([DB * P, TQ], FP8, tag="kl_d", name="kl_d")
        vl_d = dp.tile([TQ, C], FP8, tag="vl_d", name="vl_d")
        kgA = dp.tile([4 * DB * P // 2, TQ], FP8, tag="kgA", name="kgA")
        kgB = dp.tile([4 * DB * P // 2, TQ], FP8, tag="kgB", name="kgB")
        vg_d = dp.tile([4 * TQ, C], FP8, tag="vg_d", name="vg_d")

        # ---- persistent SBUF (right side) ----
        qo_cm = tc.tile_pool(name="qopool", bufs=1, side="right")
        qop = qo_cm.__enter__()
        qT3 = qop.tile([P, DB, TQ], FP8, tag="qT", name="qT3")          # 8KB
        oT3 = qT3  # o^T reuses q^T: each (head, ti) slice is dead after scores

        pbA_cm = tc.tile_pool(name="pbA", bufs=1, side="right")
        pbA = pbA_cm.__enter__()
        wq3 = pbA.tile([P, CT, C], FP8, tag="wq3", name="wq3")
        wk3 = pbA.tile([P, CT, C], FP8, tag="wk3", name="wk3")
        wv3 = pbA.tile([P, CT, C], FP8, tag="wv3", name="wv3")
        aT3 = pbA.tile([P, CT, TQ], FP8, tag="aT3", name="aT3")

        pbps_cm = tc.tile_pool(name="pb_ps", bufs=1, space="PSUM")
        pbps = pbps_cm.__enter__()

        kT_cm = tc.tile_pool(name="kTpool", bufs=1)
        kTp = kT_cm.__enter__()
        kT3 = kTp.tile([P, DB, T], FP8, tag="kT", name="kT3")           # 32KB
        vB_cm = tc.tile_pool(name="vBpool", bufs=1)
        vBp = vB_cm.__enter__()
        vB3 = vBp.tile([P, TJ, C], FP8, tag="vB", name="vB3")           # 32KB

        # ---- chunk rmsnorm -> aT3 (fp8) ----
        pbN_cm = tc.tile_pool(name="pbN", bufs=1)
        pbN = pbN_cm.__enter__()
        xts = []
        for t2 in range(TQT):
            xt = pbN.tile([P, CT, NT], BF16, tag="xqt", bufs=2, name="xqt")
            for ci in range(CT):
                nc.sync.dma_start(xt[:, ci, :],
                                  xqb[ci * P:(ci + 1) * P, t2 * NT:(t2 + 1) * NT])
            xts.append(xt)
        for ci in range(CT):
            nc.sync.dma_start(wk3[:, ci, :], wk[ci * P:(ci + 1) * P, :])
            nc.sync.dma_start(wq3[:, ci, :], wq[ci * P:(ci + 1) * P, :])
            nc.sync.dma_start(wv3[:, ci, :], wv[ci * P:(ci + 1) * P, :])
        for t2 in range(TQT):
            xt = xts[t2]
            sq3 = pbN.tile([P, CT, NT], FP8, tag="sqb", bufs=2, name="sqb")
            for cp_ in range(CT // 2):
                eng = nc.vector if cp_ % 2 == 0 else nc.gpsimd
                eng.tensor_mul(sq3[:, 2 * cp_:2 * cp_ + 2, :],
                               xt[:, 2 * cp_:2 * cp_ + 2, :],
                               xt[:, 2 * cp_:2 * cp_ + 2, :])
            ss = pbps.tile([P, NT], F32, tag="ssb", bufs=2, name="ssb")
            for cp_ in range(CT // 2):
                nc.tensor.matmul(ss[:], ones8[:], sq3[:, 2 * cp_:2 * cp_ + 2, :],
                                 start=(cp_ == 0), stop=(cp_ == CT // 2 - 1),
                                 perf_mode=DRW)
            sqt = pbN.tile([P, NT], F32, tag="sqtb", bufs=2, name="sqtb")
            nc.scalar.activation(sqt[:], ss[:], AF.Sqrt, scale=1.0 / C, bias=eps_t[:])
            rn = pbN.tile([P, NT], F32, tag="rnb", bufs=2, name="rnb")
            nc.vector.reciprocal(rn[:], sqt[:])
            for ci in range(CT):
                eng = nc.vector if ci % 2 == 0 else nc.gpsimd
                eng.tensor_mul(aT3[:, ci, t2 * NT:(t2 + 1) * NT],
                               xt[:, ci, :], rn[:])

        # ---- K/V chunks -> DRAM -> interleaved AllGathers ----
        vgA = dp.tile([4 * TQ // 2, C], FP8, tag="vgA", name="vgA")
        vgB = dp.tile([4 * TQ // 2, C], FP8, tag="vgB", name="vgB")

        def k_block(dp_lo, dp_hi):
            for dp_ in range(dp_lo, dp_hi):
                for t2 in range(TQT):
                    pk = pbps.tile([P, 2, NT], F32, tag="pp", bufs=3, name="pk")
                    for half in range(2):
                        db = 2 * dp_ + half
                        for cp_ in range(CT // 2):
                            nc.tensor.matmul(
                                pk[:, half, :],
                                wk3[:, 2 * cp_:2 * cp_ + 2, db * P:(db + 1) * P],
                                aT3[:, 2 * cp_:2 * cp_ + 2, t2 * NT:(t2 + 1) * NT],
                                start=(cp_ == 0), stop=(cp_ == CT // 2 - 1),
                                perf_mode=DRW)
                    st = pbN.tile([P, 2, NT], FP8, tag="stk", bufs=3, name="stk")
                    nc.scalar.copy(st[:], pk[:])
                    for half in range(2):
                        nc.sync.dma_start(
                            kl_d[(2 * dp_ + half) * P:(2 * dp_ + half + 1) * P,
                                 t2 * NT:(t2 + 1) * NT],
                            st[:, half, :])

        def v_block(jl_lo, jl_hi):
            for jl in range(jl_lo, jl_hi):
                pv = pbps.tile([P, 2, NT], F32, tag="pp", bufs=3, name="pv")
                for hf in range(2):
                    for cp_ in range(CT // 2):
                        nc.tensor.matmul(
                            pv[:, hf, :],
                            aT3[:, 2 * cp_:2 * cp_ + 2, jl * P:(jl + 1) * P],
                            wv3[:, 2 * cp_:2 * cp_ + 2, hf * NT:(hf + 1) * NT],
                            start=(cp_ == 0), stop=(cp_ == CT // 2 - 1),
                            perf_mode=DRW)
                st = pbN.tile([P, 2, NT], FP8, tag="stv", bufs=3, name="stv")
                nc.scalar.copy(st[:], pv[:])
                nc.sync.dma_start(vl_d[jl * P:(jl + 1) * P, :], st[:])

        HTQ = TQ // 2
        k_block(0, DB // 4)
        nc.gpsimd.collective_compute(
            "AllGather", mybir.AluOpType.bypass, replica_groups=RG,
            ins=[kl_d[0:DB * P // 2, :]], outs=[kgA[:, :]])
        v_block(0, TQ // P // 2)
        nc.gpsimd.collective_compute(
            "AllGather", mybir.AluOpType.bypass, replica_groups=RG,
            ins=[vl_d[0:HTQ, :]], outs=[vgA[:, :]])
        v_block(TQ // P // 2, TQ // P)
        nc.gpsimd.collective_compute(
            "AllGather", mybir.AluOpType.bypass, replica_groups=RG,
            ins=[vl_d[HTQ:TQ, :]], outs=[vgB[:, :]])
        k_block(DB // 4, DB // 2)
        nc.gpsimd.collective_compute(
            "AllGather", mybir.AluOpType.bypass, replica_groups=RG,
            ins=[kl_d[DB * P // 2:DB * P, :]], outs=[kgB[:, :]])

        # loads: kT-A, vB-A, vB-B, then kT-B (kT-B needed latest)
        HDB = DB * P // 2
        for db in range(DB // 2):
            for r in range(4):
                nc.sync.dma_start(
                    kT3[:, db, r * TQ:(r + 1) * TQ],
                    kgA[r * HDB + db * P: r * HDB + (db + 1) * P, :])
        for r in range(4):
            for jl in range(TQ // P // 2):
                nc.sync.dma_start(
                    vB3[:, r * (TQ // P) + jl, :],
                    vgA[r * HTQ + jl * P: r * HTQ + (jl + 1) * P, :])
        for r in range(4):
            for jl in range(TQ // P // 2, TQ // P):
                jbl = jl - TQ // P // 2
                nc.sync.dma_start(
                    vB3[:, r * (TQ // P) + jl, :],
                    vgB[r * HTQ + jbl * P: r * HTQ + (jbl + 1) * P, :])
        for db in range(DB // 2, DB):
            dbl = db - DB // 2
            for r in range(4):
                nc.sync.dma_start(
                    kT3[:, db, r * TQ:(r + 1) * TQ],
                    kgB[r * HDB + dbl * P: r * HDB + (dbl + 1) * P, :])

        # ---- Q ----
        for t2 in range(TQT):
            for dp_ in range(DB // 2):
                pq = pbps.tile([P, 2, NT], F32, tag="pp", bufs=3, name="pq")
                for half in range(2):
                    db = 2 * dp_ + half
                    for cp_ in range(CT // 2):
                        nc.tensor.matmul(
                            pq[:, half, :],
                            wq3[:, 2 * cp_:2 * cp_ + 2, db * P:(db + 1) * P],
                            aT3[:, 2 * cp_:2 * cp_ + 2, t2 * NT:(t2 + 1) * NT],
                            start=(cp_ == 0), stop=(cp_ == CT // 2 - 1),
                            perf_mode=DRW)
                qpair = qT3[:, 2 * dp_:2 * dp_ + 2, t2 * NT:(t2 + 1) * NT]
                nc.scalar.copy(qpair, pq[:])
        pbN_cm.__exit__(None, None, None)
        pbps_cm.__exit__(None, None, None)
        pbA_cm.__exit__(None, None, None)

        if DEBUG:
            nc.sync.dma_start(dbg_q[:, :], qT3[:, :, :])
            nc.sync.dma_start(dbg_k[:, :], kT3[:, :, :])
            nc.sync.dma_start(dbg_v[:, :], vB3[:, :, :])

        # ---------------- attention (+ interleaved Wo/residual) ----------------
        hR_cm = tc.tile_pool(name="hpool", bufs=1, side="right")
        hRp = hR_cm.__enter__()
        hB = hRp.tile([P, CT, TQ], F32, tag="hB", name="hB")            # 32KB
        wo_cm = tc.tile_pool(name="wopool", bufs=1, side="right")
        wop = wo_cm.__enter__()
        wo3 = wop.tile([P, CT, C], FP8, tag="wo3", name="wo3")
        for ci in range(CT):
            nc.sync.dma_start(wo3[:, ci, :], wo[ci * P:(ci + 1) * P, :])
        pdX_cm = tc.tile_pool(name="pdX", bufs=1)
        pdXp = pdX_cm.__enter__()
        xqD = pdXp.tile([P, CT, TQ], F32, tag="xqD", name="xqD")        # 32KB
        for ci in range(CT):
            nc.sync.dma_start(xqD[:, ci, :], xq[ci * P:(ci + 1) * P, :])

        pc_cm = tc.tile_pool(name="pc", bufs=1)
        pcp = pc_cm.__enter__()
        pss_cm = tc.tile_pool(name="ps_s", bufs=2, space="PSUM")
        pss = pss_cm.__enter__()
        pso_cm = tc.tile_pool(name="ps_o", bufs=1, space="PSUM")
        pso = pso_cm.__enter__()
        for ti in range(TQT):
            for h in range(H):
                et3 = pcp.tile([P, TJ, NT], FP8, tag="exp", bufs=2, name="et3")
                q_sl = qT3[:, 2 * h:2 * h + 2, ti * NT:(ti + 1) * NT]
                for jp in range(JP):
                    psc = pss.tile([P, 2, NT], F32, tag="s", bufs=2, name="psc")
                    for half in range(2):
                        tj = 2 * jp + half
                        nc.tensor.matmul(
                            psc[:, half, :],
                            kT3[:, 2 * h:2 * h + 2, tj * P:(tj + 1) * P],
                            q_sl, start=True, stop=True, perf_mode=DRW)
                    e_sl = et3[:, 2 * jp:2 * jp + 2, :]
                    if jp >= JP - SCH_PAIRS and not (ti == 0 and h <= 2):
                        sch = pcp.tile([P, 2, NT], I32, tag="sch", bufs=3,
                                       name="sch")
                        nc.vector.tensor_scalar(sch[:], psc[:],
                                                SCH_A * EXP_SCALE, SCH_B,
                                                ALU.mult, ALU.add)
                        nc.gpsimd.tensor_copy(e_sl, sch[:].bitcast(F32))
                    else:
                        nc.scalar.activation(e_sl, psc[:], AF.Exp,
                                             scale=EXP_SCALE)
                po0 = pso.tile([P, NT], F32, tag="po0", name="po0")
                po1 = pso.tile([P, NT], F32, tag="po1", name="po1")
                pr = pso.tile([P, NT], F32, tag="pr", name="pr")
                for jp in range(JP):
                    e_sl = et3[:, 2 * jp:2 * jp + 2, :]
                    st_, sp_ = (jp == 0), (jp == JP - 1)
                    nc.tensor.matmul(pr[:], ones_d[:], e_sl, start=st_, stop=sp_,
                                     perf_mode=DRW, skip_group_check=True)
                    nc.tensor.matmul(po0[:],
                                     vB3[:, 2 * jp:2 * jp + 2, h * DH: h * DH + P],
                                     e_sl, start=st_, stop=sp_,
                                     perf_mode=DRW, skip_group_check=True)
                    nc.tensor.matmul(po1[:],
                                     vB3[:, 2 * jp:2 * jp + 2, h * DH + P:(h + 1) * DH],
                                     e_sl, start=st_, stop=sp_,
                                     perf_mode=DRW, skip_group_check=True)
                rec = pcp.tile([P, NT], F32, tag="rec", bufs=2, name="rec")
                nc.vector.reciprocal(rec[:], pr[:])
                if DEBUG and ti == 0 and h == 0:
                    nc.sync.dma_start(dbg_e[:, :], et3[:, :, :])
                    prs = pcp.tile([P, NT], F32, tag="prs", bufs=1, name="prs")
                    nc.vector.tensor_copy(prs[:], pr[:])
                    nc.sync.dma_start(dbg_r[:, :], prs[:])
                nc.vector.tensor_mul(oT3[:, 2 * h, ti * NT:(ti + 1) * NT],
                                     po0[:], rec[:])
                nc.vector.tensor_mul(oT3[:, 2 * h + 1, ti * NT:(ti + 1) * NT],
                                     po1[:], rec[:])
            # ---- Wo + residual for this token half (hides inside attention) ----
            t2 = ti
            for cb in range(CT):
                ph = pso.tile([P, NT], F32, tag="ph", bufs=1, name="ph")
                for cp_ in range(CT // 2):
                    nc.tensor.matmul(
                        ph[:],
                        wo3[:, 2 * cp_:2 * cp_ + 2, cb * P:(cb + 1) * P],
                        oT3[:, 2 * cp_:2 * cp_ + 2, t2 * NT:(t2 + 1) * NT],
                        start=(cp_ == 0), stop=(cp_ == CT // 2 - 1),
                        perf_mode=DRW)
                nc.vector.scalar_tensor_tensor(
                    hB[:, cb, t2 * NT:(t2 + 1) * NT], ph[:],
                    1.0 / (OSC * WS), xqD[:, cb, t2 * NT:(t2 + 1) * NT],
                    ALU.mult, ALU.add)
        pso_cm.__exit__(None, None, None)
        pss_cm.__exit__(None, None, None)
        pc_cm.__exit__(None, None, None)
        pdX_cm.__exit__(None, None, None)
        vB_cm.__exit__(None, None, None)
        kT_cm.__exit__(None, None, None)

        if DEBUG:
            nc.sync.dma_start(dbg_o[:, :], oT3[:, :, :])

        wo_cm.__exit__(None, None, None)

        # ---------------- FFN ----------------
        pe_cm = tc.tile_pool(name="pe", bufs=1)
        pep = pe_cm.__enter__()
        peps_cm = tc.tile_pool(name="pe_ps", bufs=2, space="PSUM")
        peps = peps_cm.__enter__()
        w13 = pep.tile([P, CT, FF], FP8, tag="w13", name="w13")         # 12KB
        for ci in range(CT):
            nc.sync.dma_start(w13[:, ci, :], w1[ci * P:(ci + 1) * P, :])
        w23 = pep.tile([P, FFB, C], BF16, tag="w23", name="w23")        # 24KB
        for fi in range(FFB):
            nc.sync.dma_start(w23[:, fi, :], w2[fi * P:(fi + 1) * P, :])
        fB3 = pep.tile([P, CT, TQ], FP8, tag="fB3", name="fB3")         # 8KB
        gB3 = pep.tile([P, FFB, TQ], BF16, tag="gB3", name="gB3")       # 24KB
        for t2 in range(TQT):
            sq3 = pep.tile([P, CT, NT], FP8, tag="sqe", bufs=2, name="sqe")
            for cp_ in range(CT // 2):
                eng = nc.gpsimd if cp_ % 2 == 0 else nc.vector
                eng.tensor_mul(sq3[:, 2 * cp_:2 * cp_ + 2, :],
                               hB[:, 2 * cp_:2 * cp_ + 2, t2 * NT:(t2 + 1) * NT],
                               hB[:, 2 * cp_:2 * cp_ + 2, t2 * NT:(t2 + 1) * NT])
            ss = peps.tile([P, NT], F32, tag="sse", bufs=1, name="sse")
            for cp_ in range(CT // 2):
                nc.tensor.matmul(ss[:], ones8[:], sq3[:, 2 * cp_:2 * cp_ + 2, :],
                                 start=(cp_ == 0), stop=(cp_ == CT // 2 - 1),
                                 perf_mode=DRW)
            sqt = pep.tile([P, NT], F32, tag="sqte", bufs=2, name="sqte")
            nc.scalar.activation(sqt[:], ss[:], AF.Sqrt, scale=1.0 / C, bias=eps_t[:])
            rn = pep.tile([P, NT], F32, tag="rne", bufs=2, name="rne")
            nc.vector.reciprocal(rn[:], sqt[:])
            for ci in range(CT):
                eng = nc.gpsimd if ci % 2 == 0 else nc.vector
                eng.tensor_mul(fB3[:, ci, t2 * NT:(t2 + 1) * NT],
                               hB[:, ci, t2 * NT:(t2 + 1) * NT], rn[:])
        for t2 in range(TQT):
            for fp_ in range(FFB // 2):
                pu = peps.tile([P, 2, NT], F32, tag="pu", bufs=2, name="pu")
                for half in range(2):
                    fb = 2 * fp_ + half
                    for cp_ in range(CT // 2):
                        nc.tensor.matmul(
                            pu[:, half, :],
                            w13[:, 2 * cp_:2 * cp_ + 2, fb * P:(fb + 1) * P],
                            fB3[:, 2 * cp_:2 * cp_ + 2, t2 * NT:(t2 + 1) * NT],
                            start=(cp_ == 0), stop=(cp_ == CT // 2 - 1),
                            perf_mode=DRW)
                nc.scalar.activation(
                    gB3[:, 2 * fp_:2 * fp_ + 2, t2 * NT:(t2 + 1) * NT],
                    pu[:], AF.Gelu, scale=1.0 / WS)
        for t2 in range(TQT):
            for cb in range(CT):
                py = peps.tile([P, NT], F32, tag="py", bufs=3, name="py")
                for fb in range(FFB):
                    nc.tensor.matmul(
                        py[:],
                        w23[:, fb, cb * P:(cb + 1) * P],
                        gB3[:, fb, t2 * NT:(t2 + 1) * NT],
                        start=(fb == 0), stop=(fb == FFB - 1))
                yt = pep.tile([P, NT], F32, tag="yt", bufs=3, name="yt")
                nc.vector.tensor_add(
                    yt[:], py[:], hB[:, cb, t2 * NT:(t2 + 1) * NT])
                nc.sync.dma_start(out[cb * P:(cb + 1) * P, t2 * NT:(t2 + 1) * NT],
                                  yt[:])
        peps_cm.__exit__(None, None, None)
        pe_cm.__exit__(None, None, None)
        hR_cm.__exit__(None, None, None)
        qo_cm.__exit__(None, None, None)
        dram_cm.__exit__(None, None, None)
        cpool_cm.__exit__(None, None, None)

        sched_state, snap = tc.schedule_and_allocate()
        _CACHE["predicted_ns"] = snap.time if snap is not None else None
        try:
            _CACHE["dispatch_ns"] = sched_state.get_inst_dispatch_ns()
        except Exception:
            _CACHE["dispatch_ns"] = None

    nc.finalize()
    return nc


def get_nc():
    if "nc" not in _CACHE:
        _CACHE["nc"] = _build()
    return _CACHE["nc"]


def _prep_inputs(inputs):
    f8 = ml_dtypes.float8_e4m3
    x = np.asarray(inputs["x"], dtype=np.float32)
    g_attn = np.asarray(inputs["g_attn"], np.float32)
    g_ff = np.asarray(inputs["g_ff"], np.float32)
    wq8 = (g_attn[:, None] * np.asarray(inputs["Wq"], np.float32) * WS_QKV).astype(f8)
    wk8 = (g_attn[:, None] * np.asarray(inputs["Wk"], np.float32) * WS_QKV).astype(f8)
    wv8 = (g_attn[:, None] * np.asarray(inputs["Wv"], np.float32) * WS_QKV).astype(f8)
    wo8 = (np.asarray(inputs["Wo"], np.float32) * WS).astype(f8)
    w18 = (g_ff[:, None] * np.asarray(inputs["W1"], np.float32) * WS).astype(f8)
    w28 = np.asarray(inputs["W2"], np.float32).astype(ml_dtypes.bfloat16)
    in_maps = []
    for core in range(8):
        b, cq = divmod(core, 4)
        xc = np.ascontiguousarray(x[b][:, cq * TQ:(cq + 1) * TQ])
        in_maps.append({
            "xq": xc,
            "xqb": xc.astype(ml_dtypes.bfloat16),
            "wq": wq8, "wk": wk8, "wv": wv8, "wo": wo8, "w1": w18, "w2": w28,
        })
    return in_maps


def run(inputs, **kwargs):
    nc = get_nc()
    in_maps = _prep_inputs(inputs)
    res = run_bass_kernel_spmd(nc, in_maps, core_ids=list(range(8)), **kwargs)
    out = np.empty((B, C, T), np.float32)
    for core in range(8):
        b, cq = divmod(core, 4)
        out[b][:, cq * TQ:(cq + 1) * TQ] = res.results[core]["out"]
    return out, res


def kernel(**inputs) -> np.ndarray:
    out, _ = run(inputs)
    return out
